# revision 4
# baseline (speedup 1.0000x reference)
"""Trainium2 Bass kernel for DemoDynamicTokenSorterV2 (topk_masking).

Computation: out = concat_b(T @ hidden[b]) where T (K x N) is a shared
token-selection/merge matrix derived from keep_prob_template:
  - keep rows (p >= 0.6): one-hot -> pure row gather
  - merge rows: weighted segment-sum of merge tokens (0.2 <= p < 0.6) into
    grid clusters.

Strategy: data-parallel over batch B=8 across 8 NeuronCores (one sample per
core, T shared). T is never materialized densely on device:
  - keep rows: dma_gather (HBM->SBUF) + contiguous store (SBUF->HBM)
  - merge rows: tokens sorted by cluster, chunked by 128; each chunk's
    weights form a narrow block of T loaded as the stationary matmul
    operand; PE accumulates all chunks of a 128-cluster "half" into PSUM.

The index plan is computed on host from keep_prob_template (mirroring the
reference's numpy `plan()`), baked into the traced program as int16 gather
tables and packed per-chunk weight blocks.
"""

import math
import os

import numpy as np

import concourse.bass as bass
import concourse.tile as tile
from concourse import bacc, mybir
from concourse import bass_utils

B, TG, HG, WG, D = 8, 4, 40, 40, 2048
N = TG * HG * WG  # 6400
PRUNE_LOW, KEEP_HIGH = 0.2, 0.6
MERGE_DIVISOR, MAX_MERGE_RATIO, MIN_MERGE_TOKENS, EPS = 10, 0.05, 8, 1e-6

F32 = mybir.dt.float32
I16 = mybir.dt.int16

N_CORES = 8
GROUP_TOKENS = 512          # tokens per dma_gather call (4 chunks of 128)
PSUM_FREE = 512             # fp32 psum bank free-dim

# module-level cache: template bytes -> (nc, meta)
_CACHE = {}

# results of the last device run (for test harness inspection)
LAST_RESULTS = None


def _plan(p_np):
    """Mirror reference.plan(): keep indices, merge indices, cluster
    assignment (compacted), #clusters."""
    keep_idx = np.nonzero(p_np >= KEEP_HIGH)[0]
    merge_idx = np.nonzero((p_np >= PRUNE_LOW) & (p_np < KEEP_HIGH))[0]
    m = int(merge_idx.size)
    if m < MIN_MERGE_TOKENS:
        return keep_idx, merge_idx[:0], np.zeros(0, np.int64), 0
    target = min(
        max(1, math.ceil(N * MAX_MERGE_RATIO)),
        max(1, math.ceil(m / MERGE_DIVISOR)),
        m,
    )
    best_prod, best = 1, (1, 1, 1)
    for nt in range(1, TG + 1):
        for nh in range(1, HG + 1):
            max_nw = min(WG, target // (nt * nh))
            if max_nw < 1:
                continue
            prod = nt * nh * max_nw
            if prod > best_prod:
                best_prod, best = prod, (nt, nh, max_nw)
    nt, nh, nw = best
    coords = np.stack(
        np.meshgrid(np.arange(TG), np.arange(HG), np.arange(WG), indexing="ij"), -1
    ).reshape(-1, 3)[:N]
    cm = coords[merge_idx]
    bt = np.minimum(cm[:, 0] * nt // TG, nt - 1)
    bh = np.minimum(cm[:, 1] * nh // HG, nh - 1)
    bw = np.minimum(cm[:, 2] * nw // WG, nw - 1)
    cid = bt * (nh * nw) + bh * nw + bw
    uniq, mapped = np.unique(cid, return_inverse=True)
    return keep_idx, merge_idx, mapped, int(uniq.size)


def _round_up(x, m):
    return (x + m - 1) // m * m


def _wrap_idx_table(idx, pad_to):
    """int16 gather-index table: flat index i lives at [i % 16, i // 16],
    replicated across the 8 groups of 16 partitions."""
    a = np.zeros(pad_to, np.int16)
    a[: len(idx)] = idx.astype(np.int16)
    # padding entries point at row 0 (valid; results discarded/zero-weighted)
    w = a.reshape(pad_to // 16, 16).T.copy()  # [16, pad/16]
    return np.ascontiguousarray(np.tile(w, (8, 1)))  # [128, pad/16]


def _prepare(template_np):
    """Host-side plan + device-program metadata from the keep-prob template."""
    p_raw = np.asarray(template_np, np.float32)[:N]
    keep_idx, merge_idx, mapped, C = _plan(p_raw)
    nk = int(keep_idx.size)

    p = np.clip(p_raw, np.float32(1e-6), np.float32(1 - 1e-6))
    if C > 0:
        w = np.clip(
            (p[merge_idx] - np.float32(PRUNE_LOW))
            / (np.float32(KEEP_HIGH) - np.float32(PRUNE_LOW)),
            np.float32(0.0),
            np.float32(1.0),
        ).astype(np.float32)
        denom = np.maximum(
            np.bincount(mapped, weights=w.astype(np.float64), minlength=C).astype(
                np.float32
            ),
            np.float32(EPS),
        )
        wn = (w / denom[mapped]).astype(np.float32)
    else:
        wn = np.zeros(0, np.float32)

    # ---- keep side ----
    nk_pad = max(_round_up(nk, 128), 128)
    ktab = _wrap_idx_table(keep_idx, nk_pad)

    # ---- merge side: sort tokens by cluster, pad each 128-cluster half to a
    # multiple of 128 tokens so every chunk's clusters live in one half ----
    n_halves = (C + 127) // 128
    tok_sorted = []   # original token index per padded slot
    w_sorted = []     # normalized weight per padded slot
    cl_sorted = []    # cluster id per padded slot
    if C > 0:
        order = np.argsort(mapped, kind="stable")
        for hf in range(n_halves):
            lo, hi = hf * 128, min((hf + 1) * 128, C)
            sel = order[(mapped[order] >= lo) & (mapped[order] < hi)]
            t = merge_idx[sel].astype(np.int64)
            ww = wn[sel]
            cc = mapped[sel]
            pad = _round_up(max(len(t), 1), 128) - len(t)
            tok_sorted.append(np.concatenate([t, np.zeros(pad, np.int64)]))
            w_sorted.append(np.concatenate([ww, np.zeros(pad, np.float32)]))
            cl_sorted.append(np.concatenate([cc, np.full(pad, lo, np.int64)]))
    half_chunks = [len(t) // 128 for t in tok_sorted]  # chunks per half
    n_chunks = sum(half_chunks)
    if n_chunks:
        tok_all = np.concatenate(tok_sorted)
        w_all = np.concatenate(w_sorted)
        cl_all = np.concatenate(cl_sorted)
        m_pad = len(tok_all)
        mtab = _wrap_idx_table(tok_all, m_pad)
        # packed stationary weight blocks: [128, n_chunks*128];
        # wblk[p, i*128 + c] = weight of token i*128+p toward cluster
        # half(i)*128 + c
        wblk = np.zeros((128, n_chunks * 128), np.float32)
        chunk_half = []
        ci = 0
        for hf, hc in enumerate(half_chunks):
            for _ in range(hc):
                chunk_half.append(hf)
                ci += 1
        for i in range(n_chunks):
            hf = chunk_half[i]
            for pp in range(128):
                j = i * 128 + pp
                cloc = int(cl_all[j]) - hf * 128
                if w_all[j] != 0.0:
                    wblk[pp, i * 128 + cloc] = w_all[j]
    else:
        m_pad = 0
        mtab = np.zeros((128, 1), np.int16)
        wblk = np.zeros((128, 128), np.float32)
        chunk_half = []

    meta = dict(
        nk=nk, nk_pad=nk_pad, C=C, n_halves=n_halves,
        n_chunks=n_chunks, chunk_half=chunk_half, half_chunks=half_chunks,
        m_pad=m_pad, K=nk + C,
    )
    return meta, ktab, mtab, wblk


def _build(meta, SK, SM):
    """Trace + compile the per-core Bass program."""
    nk_pad = meta["nk_pad"]
    n_halves = meta["n_halves"]
    n_chunks = meta["n_chunks"]
    chunk_half = meta["chunk_half"]
    half_chunks = meta["half_chunks"]

    nc = bacc.Bacc("TRN2", target_bir_lowering=False, debug=False)
    h = nc.dram_tensor("h", [N, D], F32, kind="ExternalInput").ap()
    kidx = nc.dram_tensor("kidx", [128, SK], I16, kind="ExternalInput").ap()
    midx = nc.dram_tensor("midx", [128, SM], I16, kind="ExternalInput").ap()
    wblk_cols = max(n_chunks, 1) * 128
    wblk = nc.dram_tensor("wblk", [128, wblk_cols], F32, kind="ExternalInput").ap()
    R = nk_pad + n_halves * 128
    out = nc.dram_tensor("out", [R, D], F32, kind="ExternalOutput").ap()

    # keep rows of `out`, partition-major view for SBUF-shaped stores
    out_keep = out[0:nk_pad, :].rearrange("(a p) e -> p a e", p=128)

    # keep groups: [start_chunk, n_chunks_in_group]
    kg = []
    total_kc = nk_pad // 128
    c0 = 0
    while c0 < total_kc:
        kc = min(GROUP_TOKENS // 128, total_kc - c0)
        kg.append((c0, kc))
        c0 += kc
    # merge groups: [start_chunk, n_chunks_in_group] never crossing a half
    mg = []
    ci = 0
    for hf, hc in enumerate(half_chunks):
        left = hc
        while left > 0:
            gc = min(GROUP_TOKENS // 128, left)
            mg.append((ci, gc))
            ci += gc
            left -= gc

    with tile.TileContext(nc) as tc:
        with (
            tc.tile_pool(name="const", bufs=1) as cpool,
            tc.tile_pool(name="keep", bufs=2) as kpool,
            tc.tile_pool(name="merge", bufs=2) as mpool,
            tc.tile_pool(name="stage", bufs=2) as spool,
            tc.tile_pool(name="psum", bufs=1,
                         space=bass.MemorySpace.PSUM) as ppool,
        ):
            kidx_sb = cpool.tile([128, SK], I16, tag="kidx")
            nc.sync.dma_start(kidx_sb[:], kidx[:])
            midx_sb = cpool.tile([128, SM], I16, tag="midx")
            nc.sync.dma_start(midx_sb[:], midx[:])
            wblk_sb = cpool.tile([128, wblk_cols], F32, tag="wblk")
            nc.sync.dma_start(wblk_sb[:], wblk[:])

            psum_tiles = {}
            for hf in range(n_halves):
                for dk in range(D // PSUM_FREE):
                    psum_tiles[(hf, dk)] = ppool.tile(
                        [128, PSUM_FREE], F32,
                        name=f"ps{hf}_{dk}", tag=f"ps{hf}_{dk}",
                    )

            def emit_keep_group(g):
                gc0, gcn = kg[g]
                ntok = gcn * 128
                kt = kpool.tile([128, GROUP_TOKENS // 128, D], F32, tag="kt")
                nc.gpsimd.dma_gather(
                    kt[:, 0:gcn, :], h[:], kidx_sb[:, gc0 * 8 : gc0 * 8 + ntok // 16],
                    ntok, ntok, D, elem_step=D,
                )
                nc.sync.dma_start(out_keep[:, gc0 : gc0 + gcn, :], kt[:, 0:gcn, :])

            done_in_half = [0] * max(n_halves, 1)

            def emit_merge_group(g):
                gc0, gcn = mg[g]
                ntok = gcn * 128
                hf = chunk_half[gc0]
                mt = mpool.tile([128, GROUP_TOKENS // 128, D], F32, tag="mt")
                nc.gpsimd.dma_gather(
                    mt[:, 0:gcn, :], h[:], midx_sb[:, gc0 * 8 : gc0 * 8 + ntok // 16],
                    ntok, ntok, D, elem_step=D,
                )
                for cc in range(gcn):
                    i = gc0 + cc
                    first = done_in_half[hf] == 0
                    done_in_half[hf] += 1
                    last = done_in_half[hf] == half_chunks[hf]
                    for dk in range(D // PSUM_FREE):
                        nc.tensor.matmul(
                            psum_tiles[(hf, dk)][:],
                            wblk_sb[:, i * 128 : (i + 1) * 128],
                            mt[:, cc, dk * PSUM_FREE : (dk + 1) * PSUM_FREE],
                            start=first, stop=last,
                        )
                if done_in_half[hf] == half_chunks[hf]:
                    stage = spool.tile([128, D], F32, tag="stage")
                    for dk in range(D // PSUM_FREE):
                        nc.vector.tensor_copy(
                            stage[:, dk * PSUM_FREE : (dk + 1) * PSUM_FREE],
                            psum_tiles[(hf, dk)][:],
                        )
                    r0 = nk_pad + hf * 128
                    nc.sync.dma_start(out[r0 : r0 + 128, :], stage[:])

            # interleave merge (PE) and keep (pure DMA) work
            for step in range(max(len(kg), len(mg))):
                if step < len(mg):
                    emit_merge_group(step)
                if step < len(kg):
                    emit_keep_group(step)

    nc.compile()
    return nc


def _get_program(template_np):
    key = np.asarray(template_np, np.float32).tobytes()
    if key not in _CACHE:
        meta, ktab, mtab, wblk = _prepare(template_np)
        nc = _build(meta, ktab.shape[1], mtab.shape[1])
        _CACHE[key] = (nc, meta, ktab, mtab, wblk)
    return _CACHE[key]


def kernel(hidden_states, lengths, keep_prob_template, image_grid_thw):
    global LAST_RESULTS
    hs = np.ascontiguousarray(np.asarray(hidden_states, np.float32))
    assert hs.shape == (B, N, D), hs.shape
    nc, meta, ktab, mtab, wblk = _get_program(keep_prob_template)

    in_maps = [
        {
            "h": hs[b],
            "kidx": ktab,
            "midx": mtab,
            "wblk": wblk,
        }
        for b in range(N_CORES)
    ]
    res = bass_utils.run_bass_kernel_spmd(
        nc, in_maps, core_ids=list(range(N_CORES))
    )
    LAST_RESULTS = res

    nk, nk_pad, C, n_halves = meta["nk"], meta["nk_pad"], meta["C"], meta["n_halves"]
    outs = []
    for b in range(N_CORES):
        o = res.results[b]["out"]
        parts = [o[:nk]]
        for hf in range(n_halves):
            r0 = nk_pad + hf * 128
            parts.append(o[r0 : r0 + min(128, C - hf * 128)])
        outs.append(np.concatenate(parts, axis=0))
    return np.concatenate(outs, axis=0)


# revision 5
# speedup vs baseline: 1.0712x; 1.0712x over previous
"""Trainium2 Bass kernel for DemoDynamicTokenSorterV2 (topk_masking).

Computation: out = concat_b(T @ hidden[b]) where T (K x N) is a shared
token-selection/merge matrix derived from keep_prob_template:
  - keep rows (p >= 0.6): one-hot -> pure row gather
  - merge rows: weighted segment-sum of merge tokens (0.2 <= p < 0.6) into
    grid clusters.

Strategy: data-parallel over batch B=8 across 8 NeuronCores (one sample per
core, T shared). T is never materialized densely on device:
  - keep rows: dma_gather (HBM->SBUF) + contiguous store (SBUF->HBM)
  - merge rows: tokens sorted by cluster, chunked by 128; each chunk's
    weights form a narrow block of T loaded as the stationary matmul
    operand; PE accumulates all chunks of a 128-cluster "half" into PSUM.

The index plan is computed on host from keep_prob_template (mirroring the
reference's numpy `plan()`), baked into the traced program as int16 gather
tables and packed per-chunk weight blocks.
"""

import math
import os

import numpy as np

import concourse.bass as bass
import concourse.tile as tile
from concourse import bacc, mybir
from concourse import bass_utils

B, TG, HG, WG, D = 8, 4, 40, 40, 2048
N = TG * HG * WG  # 6400
PRUNE_LOW, KEEP_HIGH = 0.2, 0.6
MERGE_DIVISOR, MAX_MERGE_RATIO, MIN_MERGE_TOKENS, EPS = 10, 0.05, 8, 1e-6

F32 = mybir.dt.float32
I16 = mybir.dt.int16

N_CORES = 8
GROUP_TOKENS = 512          # tokens per dma_gather call (4 chunks of 128)
PSUM_FREE = 512             # fp32 psum bank free-dim

# module-level cache: template bytes -> (nc, meta)
_CACHE = {}

# results of the last device run (for test harness inspection)
LAST_RESULTS = None


def _plan(p_np):
    """Mirror reference.plan(): keep indices, merge indices, cluster
    assignment (compacted), #clusters."""
    keep_idx = np.nonzero(p_np >= KEEP_HIGH)[0]
    merge_idx = np.nonzero((p_np >= PRUNE_LOW) & (p_np < KEEP_HIGH))[0]
    m = int(merge_idx.size)
    if m < MIN_MERGE_TOKENS:
        return keep_idx, merge_idx[:0], np.zeros(0, np.int64), 0
    target = min(
        max(1, math.ceil(N * MAX_MERGE_RATIO)),
        max(1, math.ceil(m / MERGE_DIVISOR)),
        m,
    )
    best_prod, best = 1, (1, 1, 1)
    for nt in range(1, TG + 1):
        for nh in range(1, HG + 1):
            max_nw = min(WG, target // (nt * nh))
            if max_nw < 1:
                continue
            prod = nt * nh * max_nw
            if prod > best_prod:
                best_prod, best = prod, (nt, nh, max_nw)
    nt, nh, nw = best
    coords = np.stack(
        np.meshgrid(np.arange(TG), np.arange(HG), np.arange(WG), indexing="ij"), -1
    ).reshape(-1, 3)[:N]
    cm = coords[merge_idx]
    bt = np.minimum(cm[:, 0] * nt // TG, nt - 1)
    bh = np.minimum(cm[:, 1] * nh // HG, nh - 1)
    bw = np.minimum(cm[:, 2] * nw // WG, nw - 1)
    cid = bt * (nh * nw) + bh * nw + bw
    uniq, mapped = np.unique(cid, return_inverse=True)
    return keep_idx, merge_idx, mapped, int(uniq.size)


def _round_up(x, m):
    return (x + m - 1) // m * m


def _wrap_idx_table(idx, pad_to):
    """int16 gather-index table: flat index i lives at [i % 16, i // 16],
    replicated across the 8 groups of 16 partitions."""
    a = np.zeros(pad_to, np.int16)
    a[: len(idx)] = idx.astype(np.int16)
    # padding entries point at row 0 (valid; results discarded/zero-weighted)
    w = a.reshape(pad_to // 16, 16).T.copy()  # [16, pad/16]
    return np.ascontiguousarray(np.tile(w, (8, 1)))  # [128, pad/16]


def _prepare(template_np):
    """Host-side plan + device-program metadata from the keep-prob template."""
    p_raw = np.asarray(template_np, np.float32)[:N]
    keep_idx, merge_idx, mapped, C = _plan(p_raw)
    nk = int(keep_idx.size)

    p = np.clip(p_raw, np.float32(1e-6), np.float32(1 - 1e-6))
    if C > 0:
        w = np.clip(
            (p[merge_idx] - np.float32(PRUNE_LOW))
            / (np.float32(KEEP_HIGH) - np.float32(PRUNE_LOW)),
            np.float32(0.0),
            np.float32(1.0),
        ).astype(np.float32)
        denom = np.maximum(
            np.bincount(mapped, weights=w.astype(np.float64), minlength=C).astype(
                np.float32
            ),
            np.float32(EPS),
        )
        wn = (w / denom[mapped]).astype(np.float32)
    else:
        wn = np.zeros(0, np.float32)

    # ---- keep side ----
    nk_pad = max(_round_up(nk, 128), 128)
    ktab = _wrap_idx_table(keep_idx, nk_pad)

    # ---- merge side: sort tokens by cluster, pad each 128-cluster half to a
    # multiple of 128 tokens so every chunk's clusters live in one half ----
    n_halves = (C + 127) // 128
    tok_sorted = []   # original token index per padded slot
    w_sorted = []     # normalized weight per padded slot
    cl_sorted = []    # cluster id per padded slot
    if C > 0:
        order = np.argsort(mapped, kind="stable")
        for hf in range(n_halves):
            lo, hi = hf * 128, min((hf + 1) * 128, C)
            sel = order[(mapped[order] >= lo) & (mapped[order] < hi)]
            t = merge_idx[sel].astype(np.int64)
            ww = wn[sel]
            cc = mapped[sel]
            pad = _round_up(max(len(t), 1), 128) - len(t)
            tok_sorted.append(np.concatenate([t, np.zeros(pad, np.int64)]))
            w_sorted.append(np.concatenate([ww, np.zeros(pad, np.float32)]))
            cl_sorted.append(np.concatenate([cc, np.full(pad, lo, np.int64)]))
    half_chunks = [len(t) // 128 for t in tok_sorted]  # chunks per half
    n_chunks = sum(half_chunks)
    if n_chunks:
        tok_all = np.concatenate(tok_sorted)
        w_all = np.concatenate(w_sorted)
        cl_all = np.concatenate(cl_sorted)
        m_pad = len(tok_all)
        mtab = _wrap_idx_table(tok_all, m_pad)
        # packed stationary weight blocks: [128, n_chunks*128];
        # wblk[p, i*128 + c] = weight of token i*128+p toward cluster
        # half(i)*128 + c
        wblk = np.zeros((128, n_chunks * 128), np.float32)
        chunk_half = []
        ci = 0
        for hf, hc in enumerate(half_chunks):
            for _ in range(hc):
                chunk_half.append(hf)
                ci += 1
        for i in range(n_chunks):
            hf = chunk_half[i]
            for pp in range(128):
                j = i * 128 + pp
                cloc = int(cl_all[j]) - hf * 128
                if w_all[j] != 0.0:
                    wblk[pp, i * 128 + cloc] = w_all[j]
    else:
        m_pad = 0
        mtab = np.zeros((128, 1), np.int16)
        wblk = np.zeros((128, 128), np.float32)
        chunk_half = []

    meta = dict(
        nk=nk, nk_pad=nk_pad, C=C, n_halves=n_halves,
        n_chunks=n_chunks, chunk_half=chunk_half, half_chunks=half_chunks,
        m_pad=m_pad, K=nk + C,
    )
    return meta, ktab, mtab, wblk


def _build(meta, SK, SM):
    """Trace + compile the per-core Bass program.

    Structure: D is split into two half-passes (columns [0,1024) and
    [1024,2048)). Per half-pass, ALL merge tokens' half-rows are gathered
    into one resident SBUF tile, so merge gathers never wait on PE progress
    and the keep-path DMA stream flows uninterrupted. The PE consumes the
    resident tile chunk-by-chunk into PSUM (one bank per (dhalf, cluster
    half, 512-col quarter)).
    """
    nk_pad = meta["nk_pad"]
    n_halves = meta["n_halves"]
    n_chunks = meta["n_chunks"]
    chunk_half = meta["chunk_half"]
    half_chunks = meta["half_chunks"]

    nc = bacc.Bacc("TRN2", target_bir_lowering=False, debug=False)
    h = nc.dram_tensor("h", [N, D], F32, kind="ExternalInput").ap()
    kidx = nc.dram_tensor("kidx", [128, SK], I16, kind="ExternalInput").ap()
    midx = nc.dram_tensor("midx", [128, SM], I16, kind="ExternalInput").ap()
    wblk_cols = max(n_chunks, 1) * 128
    wblk = nc.dram_tensor("wblk", [128, wblk_cols], F32, kind="ExternalInput").ap()
    R = nk_pad + n_halves * 128
    out = nc.dram_tensor("out", [R, D], F32, kind="ExternalOutput").ap()

    # keep rows of `out`, partition-major view for SBUF-shaped stores
    out_keep = out[0:nk_pad, :].rearrange("(a p) e -> p a e", p=128)

    DH = D // 2          # 1024 columns per half-pass
    NDK = DH // PSUM_FREE  # psum banks per (dhalf, cluster-half) = 2

    # keep groups: [start_chunk, n_chunks_in_group]
    kg = []
    total_kc = nk_pad // 128
    c0 = 0
    while c0 < total_kc:
        kc = min(GROUP_TOKENS // 128, total_kc - c0)
        kg.append((c0, kc))
        c0 += kc
    # merge gather groups (within a dhalf): [start_chunk, n_chunks], groups
    # of up to 8 chunks (1024 tokens, 4MB at half-row width)
    mg = []
    c0 = 0
    while c0 < n_chunks:
        gc = min(8, n_chunks - c0)
        mg.append((c0, gc))
        c0 += gc

    with tile.TileContext(nc) as tc:
        with (
            tc.tile_pool(name="const", bufs=1) as cpool,
            tc.tile_pool(name="keep", bufs=2) as kpool,
            tc.tile_pool(name="merge", bufs=1) as mpool,
            tc.tile_pool(name="stage", bufs=2 * max(n_halves, 1)) as spool,
            tc.tile_pool(name="psum", bufs=1,
                         space=bass.MemorySpace.PSUM) as ppool,
        ):
            kidx_sb = cpool.tile([128, SK], I16, tag="kidx")
            nc.sync.dma_start(kidx_sb[:], kidx[:])
            midx_sb = cpool.tile([128, SM], I16, tag="midx")
            nc.sync.dma_start(midx_sb[:], midx[:])
            wblk_sb = cpool.tile([128, wblk_cols], F32, tag="wblk")
            nc.sync.dma_start(wblk_sb[:], wblk[:])

            mres = None
            if n_chunks:
                mres = mpool.tile([128, n_chunks, DH], F32, name="mres",
                                  tag="mres")

            psum_tiles = {}
            for dh in range(2):
                for hf in range(n_halves):
                    for dk in range(NDK):
                        psum_tiles[(dh, hf, dk)] = ppool.tile(
                            [128, PSUM_FREE], F32,
                            name=f"ps{dh}_{hf}_{dk}", tag=f"ps{dh}_{hf}_{dk}",
                        )

            def emit_keep_group(g):
                gc0, gcn = kg[g]
                ntok = gcn * 128
                kt = kpool.tile([128, GROUP_TOKENS // 128, D], F32, tag="kt")
                nc.gpsimd.dma_gather(
                    kt[:, 0:gcn, :], h[:], kidx_sb[:, gc0 * 8 : gc0 * 8 + ntok // 16],
                    ntok, ntok, D, elem_step=D,
                )
                nc.sync.dma_start(out_keep[:, gc0 : gc0 + gcn, :], kt[:, 0:gcn, :])

            def emit_merge_gather(dh, g):
                gc0, gcn = mg[g]
                ntok = gcn * 128
                nc.gpsimd.dma_gather(
                    mres[:, gc0 : gc0 + gcn, :],
                    h[:, dh * DH : (dh + 1) * DH],
                    midx_sb[:, gc0 * 8 : gc0 * 8 + ntok // 16],
                    ntok, ntok, DH, elem_step=D,
                )

            def emit_merge_compute(dh):
                ci = 0
                for hf in range(n_halves):
                    for k in range(half_chunks[hf]):
                        i = ci + k
                        for dk in range(NDK):
                            nc.tensor.matmul(
                                psum_tiles[(dh, hf, dk)][:],
                                wblk_sb[:, i * 128 : (i + 1) * 128],
                                mres[:, i, dk * PSUM_FREE : (dk + 1) * PSUM_FREE],
                                start=(k == 0), stop=(k == half_chunks[hf] - 1),
                            )
                    stage = spool.tile([128, DH], F32, tag="stage")
                    for dk in range(NDK):
                        nc.vector.tensor_copy(
                            stage[:, dk * PSUM_FREE : (dk + 1) * PSUM_FREE],
                            psum_tiles[(dh, hf, dk)][:],
                        )
                    r0 = nk_pad + hf * 128
                    # scalar (ACT) HWDGE: independent queue from keep stores
                    nc.scalar.dma_start(
                        out[r0 : r0 + 128, dh * DH : (dh + 1) * DH], stage[:]
                    )
                    ci += half_chunks[hf]

            # gpsimd queue order: merge gathers dh0, all keep gathers,
            # merge gathers dh1 (which wait on dh0 matmuls) last.
            if n_chunks:
                for g in range(len(mg)):
                    emit_merge_gather(0, g)
                emit_merge_compute(0)
            for g in range(len(kg)):
                emit_keep_group(g)
            if n_chunks:
                for g in range(len(mg)):
                    emit_merge_gather(1, g)
                emit_merge_compute(1)

    nc.compile()
    return nc


def _get_program(template_np):
    key = np.asarray(template_np, np.float32).tobytes()
    if key not in _CACHE:
        meta, ktab, mtab, wblk = _prepare(template_np)
        nc = _build(meta, ktab.shape[1], mtab.shape[1])
        _CACHE[key] = (nc, meta, ktab, mtab, wblk)
    return _CACHE[key]


def kernel(hidden_states, lengths, keep_prob_template, image_grid_thw):
    global LAST_RESULTS
    hs = np.ascontiguousarray(np.asarray(hidden_states, np.float32))
    assert hs.shape == (B, N, D), hs.shape
    nc, meta, ktab, mtab, wblk = _get_program(keep_prob_template)

    in_maps = [
        {
            "h": hs[b],
            "kidx": ktab,
            "midx": mtab,
            "wblk": wblk,
        }
        for b in range(N_CORES)
    ]
    res = bass_utils.run_bass_kernel_spmd(
        nc, in_maps, core_ids=list(range(N_CORES))
    )
    LAST_RESULTS = res

    nk, nk_pad, C, n_halves = meta["nk"], meta["nk_pad"], meta["C"], meta["n_halves"]
    outs = []
    for b in range(N_CORES):
        o = res.results[b]["out"]
        parts = [o[:nk]]
        for hf in range(n_halves):
            r0 = nk_pad + hf * 128
            parts.append(o[r0 : r0 + min(128, C - hf * 128)])
        outs.append(np.concatenate(parts, axis=0))
    return np.concatenate(outs, axis=0)


# revision 13
# speedup vs baseline: 1.0725x; 1.0011x over previous
"""Trainium2 Bass kernel for DemoDynamicTokenSorterV2 (topk_masking).

Computation: out = concat_b(T @ hidden[b]) where T (K x N) is a shared
token-selection/merge matrix derived from keep_prob_template:
  - keep rows (p >= 0.6): one-hot -> pure row gather
  - merge rows: weighted segment-sum of merge tokens (0.2 <= p < 0.6) into
    grid clusters.

Strategy: data-parallel over batch B=8 across 8 NeuronCores (one sample per
core, T shared). T is never materialized densely on device:
  - keep rows: dma_gather (HBM->SBUF) + contiguous store (SBUF->HBM)
  - merge rows: tokens sorted by cluster, chunked by 128; each chunk's
    weights form a narrow block of T loaded as the stationary matmul
    operand; PE accumulates all chunks of a 128-cluster "half" into PSUM.

The index plan is computed on host from keep_prob_template (mirroring the
reference's numpy `plan()`), baked into the traced program as int16 gather
tables and packed per-chunk weight blocks.
"""

import math
import os

import numpy as np

import concourse.bass as bass
import concourse.tile as tile
from concourse import bacc, mybir
from concourse import bass_utils

B, TG, HG, WG, D = 8, 4, 40, 40, 2048
N = TG * HG * WG  # 6400
PRUNE_LOW, KEEP_HIGH = 0.2, 0.6
MERGE_DIVISOR, MAX_MERGE_RATIO, MIN_MERGE_TOKENS, EPS = 10, 0.05, 8, 1e-6

F32 = mybir.dt.float32
I16 = mybir.dt.int16

N_CORES = 8
GROUP_TOKENS = 512          # tokens per dma_gather call (4 chunks of 128)
PSUM_FREE = 512             # fp32 psum bank free-dim
USE_FP32R = os.environ.get("KERNEL_FP32R", "0") == "1"

# module-level cache: template bytes -> (nc, meta)
_CACHE = {}

# results of the last device run (for test harness inspection)
LAST_RESULTS = None


def _plan(p_np):
    """Mirror reference.plan(): keep indices, merge indices, cluster
    assignment (compacted), #clusters."""
    keep_idx = np.nonzero(p_np >= KEEP_HIGH)[0]
    merge_idx = np.nonzero((p_np >= PRUNE_LOW) & (p_np < KEEP_HIGH))[0]
    m = int(merge_idx.size)
    if m < MIN_MERGE_TOKENS:
        return keep_idx, merge_idx[:0], np.zeros(0, np.int64), 0
    target = min(
        max(1, math.ceil(N * MAX_MERGE_RATIO)),
        max(1, math.ceil(m / MERGE_DIVISOR)),
        m,
    )
    best_prod, best = 1, (1, 1, 1)
    for nt in range(1, TG + 1):
        for nh in range(1, HG + 1):
            max_nw = min(WG, target // (nt * nh))
            if max_nw < 1:
                continue
            prod = nt * nh * max_nw
            if prod > best_prod:
                best_prod, best = prod, (nt, nh, max_nw)
    nt, nh, nw = best
    coords = np.stack(
        np.meshgrid(np.arange(TG), np.arange(HG), np.arange(WG), indexing="ij"), -1
    ).reshape(-1, 3)[:N]
    cm = coords[merge_idx]
    bt = np.minimum(cm[:, 0] * nt // TG, nt - 1)
    bh = np.minimum(cm[:, 1] * nh // HG, nh - 1)
    bw = np.minimum(cm[:, 2] * nw // WG, nw - 1)
    cid = bt * (nh * nw) + bh * nw + bw
    uniq, mapped = np.unique(cid, return_inverse=True)
    return keep_idx, merge_idx, mapped, int(uniq.size)


def _round_up(x, m):
    return (x + m - 1) // m * m


def _wrap_idx_table(idx, pad_to):
    """int16 gather-index table: flat index i lives at [i % 16, i // 16],
    replicated across the 8 groups of 16 partitions."""
    a = np.zeros(pad_to, np.int16)
    a[: len(idx)] = idx.astype(np.int16)
    # padding entries point at row 0 (valid; results discarded/zero-weighted)
    w = a.reshape(pad_to // 16, 16).T.copy()  # [16, pad/16]
    return np.ascontiguousarray(np.tile(w, (8, 1)))  # [128, pad/16]


def _prepare(template_np):
    """Host-side plan + device-program metadata from the keep-prob template."""
    p_raw = np.asarray(template_np, np.float32)[:N]
    keep_idx, merge_idx, mapped, C = _plan(p_raw)
    nk = int(keep_idx.size)

    p = np.clip(p_raw, np.float32(1e-6), np.float32(1 - 1e-6))
    if C > 0:
        w = np.clip(
            (p[merge_idx] - np.float32(PRUNE_LOW))
            / (np.float32(KEEP_HIGH) - np.float32(PRUNE_LOW)),
            np.float32(0.0),
            np.float32(1.0),
        ).astype(np.float32)
        denom = np.maximum(
            np.bincount(mapped, weights=w.astype(np.float64), minlength=C).astype(
                np.float32
            ),
            np.float32(EPS),
        )
        wn = (w / denom[mapped]).astype(np.float32)
    else:
        wn = np.zeros(0, np.float32)

    # ---- keep side ----
    nk_pad = max(_round_up(nk, 128), 128)
    ktab = _wrap_idx_table(keep_idx, nk_pad)

    # ---- merge side: sort tokens by cluster, pad each 128-cluster half to a
    # multiple of 128 tokens so every chunk's clusters live in one half ----
    n_halves = (C + 127) // 128
    tok_sorted = []   # original token index per padded slot
    w_sorted = []     # normalized weight per padded slot
    cl_sorted = []    # cluster id per padded slot
    if C > 0:
        order = np.argsort(mapped, kind="stable")
        for hf in range(n_halves):
            lo, hi = hf * 128, min((hf + 1) * 128, C)
            sel = order[(mapped[order] >= lo) & (mapped[order] < hi)]
            t = merge_idx[sel].astype(np.int64)
            ww = wn[sel]
            cc = mapped[sel]
            pad = _round_up(max(len(t), 1), 128) - len(t)
            tok_sorted.append(np.concatenate([t, np.zeros(pad, np.int64)]))
            w_sorted.append(np.concatenate([ww, np.zeros(pad, np.float32)]))
            cl_sorted.append(np.concatenate([cc, np.full(pad, lo, np.int64)]))
    half_chunks = [len(t) // 128 for t in tok_sorted]  # chunks per half
    n_chunks = sum(half_chunks)
    if n_chunks:
        tok_all = np.concatenate(tok_sorted)
        w_all = np.concatenate(w_sorted)
        cl_all = np.concatenate(cl_sorted)
        m_pad = len(tok_all)
        mtab = _wrap_idx_table(tok_all, m_pad)
        # packed stationary weight blocks: [128, n_chunks*128];
        # wblk[p, i*128 + c] = weight of token i*128+p toward cluster
        # half(i)*128 + c
        wblk = np.zeros((128, n_chunks * 128), np.float32)
        chunk_half = []
        ci = 0
        for hf, hc in enumerate(half_chunks):
            for _ in range(hc):
                chunk_half.append(hf)
                ci += 1
        for i in range(n_chunks):
            hf = chunk_half[i]
            for pp in range(128):
                j = i * 128 + pp
                cloc = int(cl_all[j]) - hf * 128
                if w_all[j] != 0.0:
                    wblk[pp, i * 128 + cloc] = w_all[j]
    else:
        m_pad = 0
        mtab = np.zeros((128, 1), np.int16)
        wblk = np.zeros((128, 128), np.float32)
        chunk_half = []

    meta = dict(
        nk=nk, nk_pad=nk_pad, C=C, n_halves=n_halves,
        n_chunks=n_chunks, chunk_half=chunk_half, half_chunks=half_chunks,
        m_pad=m_pad, K=nk + C,
    )
    return meta, ktab, mtab, wblk


def _build(meta, SK, SM):
    """Trace + compile the per-core Bass program.

    Structure: D is split into two half-passes (columns [0,1024) and
    [1024,2048)). Per half-pass, ALL merge tokens' half-rows are gathered
    into one resident SBUF tile, so merge gathers never wait on PE progress
    and the keep-path DMA stream flows uninterrupted. The PE consumes the
    resident tile chunk-by-chunk into PSUM (one bank per (dhalf, cluster
    half, 512-col quarter)).
    """
    nk_pad = meta["nk_pad"]
    n_halves = meta["n_halves"]
    n_chunks = meta["n_chunks"]
    chunk_half = meta["chunk_half"]
    half_chunks = meta["half_chunks"]

    nc = bacc.Bacc("TRN2", target_bir_lowering=False, debug=False)
    h = nc.dram_tensor("h", [N, D], F32, kind="ExternalInput").ap()
    # single combined index-table tensor: columns [0,SK) = keep table,
    # [SK, SK+SM) = merge table (>=512B per partition in one DMA)
    kmidx = nc.dram_tensor("kmidx", [128, SK + SM], I16, kind="ExternalInput").ap()
    wblk_cols = max(n_chunks, 1) * 128
    wblk = nc.dram_tensor("wblk", [128, wblk_cols], F32, kind="ExternalInput").ap()
    R = nk_pad + n_halves * 128
    out = nc.dram_tensor("out", [R, D], F32, kind="ExternalOutput").ap()

    # keep rows of `out`, partition-major view for SBUF-shaped stores
    out_keep = out[0:nk_pad, :].rearrange("(a p) e -> p a e", p=128)

    DH = D // 2          # 1024 columns per half-pass
    NDK = DH // PSUM_FREE  # psum banks per (dhalf, cluster-half) = 2

    # keep groups: [start_chunk, n_chunks_in_group]
    kg = []
    total_kc = nk_pad // 128
    c0 = 0
    while c0 < total_kc:
        kc = min(GROUP_TOKENS // 128, total_kc - c0)
        kg.append((c0, kc))
        c0 += kc
    # merge gather groups (within a dhalf): [start_chunk, n_chunks], groups
    # of up to 8 chunks (1024 tokens, 4MB at half-row width)
    mg = []
    c0 = 0
    while c0 < n_chunks:
        gc = min(8, n_chunks - c0)
        mg.append((c0, gc))
        c0 += gc

    with tile.TileContext(nc) as tc:
        with (
            tc.tile_pool(name="const", bufs=1) as cpool,
            tc.tile_pool(name="keep", bufs=2) as kpool,
            tc.tile_pool(name="merge", bufs=1) as mpool,
            tc.tile_pool(name="stage", bufs=2 * max(n_halves, 1)) as spool,
            tc.tile_pool(name="psum", bufs=1,
                         space=bass.MemorySpace.PSUM) as ppool,
        ):
            kmidx_sb = cpool.tile([128, SK + SM], I16, tag="kmidx")
            nc.sync.dma_start(kmidx_sb[:], kmidx[:])
            kidx_sb = kmidx_sb[:, 0:SK]
            midx_sb = kmidx_sb[:, SK : SK + SM]
            wblk_sb = cpool.tile([128, wblk_cols], F32, tag="wblk")
            nc.sync.dma_start(wblk_sb[:], wblk[:])

            mres = None
            if n_chunks:
                mres = mpool.tile([128, n_chunks, DH], F32, name="mres",
                                  tag="mres")

            psum_tiles = {}
            for dh in range(2):
                for hf in range(n_halves):
                    for dk in range(NDK):
                        psum_tiles[(dh, hf, dk)] = ppool.tile(
                            [128, PSUM_FREE], F32,
                            name=f"ps{dh}_{hf}_{dk}", tag=f"ps{dh}_{hf}_{dk}",
                        )

            def emit_keep_group(g):
                gc0, gcn = kg[g]
                ntok = gcn * 128
                kt = kpool.tile([128, GROUP_TOKENS // 128, D], F32, tag="kt")
                nc.gpsimd.dma_gather(
                    kt[:, 0:gcn, :], h[:], kidx_sb[:, gc0 * 8 : gc0 * 8 + ntok // 16],
                    ntok, ntok, D, elem_step=D, single_packet=False,
                )
                nc.sync.dma_start(out_keep[:, gc0 : gc0 + gcn, :], kt[:, 0:gcn, :])

            def emit_merge_gather(dh, g):
                gc0, gcn = mg[g]
                ntok = gcn * 128
                nc.gpsimd.dma_gather(
                    mres[:, gc0 : gc0 + gcn, :],
                    h[:, dh * DH : (dh + 1) * DH],
                    midx_sb[:, gc0 * 8 : gc0 * 8 + ntok // 16],
                    ntok, ntok, DH, elem_step=D, single_packet=False,
                )

            def emit_merge_compute(dh):
                ci = 0
                for hf in range(n_halves):
                    for k in range(half_chunks[hf]):
                        i = ci + k
                        for dk in range(NDK):
                            lhs = wblk_sb[:, i * 128 : (i + 1) * 128]
                            rhs = mres[:, i, dk * PSUM_FREE : (dk + 1) * PSUM_FREE]
                            if USE_FP32R:
                                lhs = lhs.bitcast(mybir.dt.float32r)
                                rhs = rhs.bitcast(mybir.dt.float32r)
                            nc.tensor.matmul(
                                psum_tiles[(dh, hf, dk)][:],
                                lhs, rhs,
                                start=(k == 0), stop=(k == half_chunks[hf] - 1),
                            )
                    stage = spool.tile([128, DH], F32, tag="stage")
                    for dk in range(NDK):
                        nc.vector.tensor_copy(
                            stage[:, dk * PSUM_FREE : (dk + 1) * PSUM_FREE],
                            psum_tiles[(dh, hf, dk)][:],
                        )
                    r0 = nk_pad + hf * 128
                    # scalar (ACT) HWDGE: independent queue from keep stores
                    nc.scalar.dma_start(
                        out[r0 : r0 + 128, dh * DH : (dh + 1) * DH], stage[:]
                    )
                    ci += half_chunks[hf]

            # gpsimd queue order: merge gathers dh0, all keep gathers,
            # merge gathers dh1 (which wait on dh0 matmuls) last.
            if n_chunks:
                for g in range(len(mg)):
                    emit_merge_gather(0, g)
                emit_merge_compute(0)
            for g in range(len(kg)):
                emit_keep_group(g)
            if n_chunks:
                for g in range(len(mg)):
                    emit_merge_gather(1, g)
                emit_merge_compute(1)

    nc.compile()
    return nc


def _get_program(template_np):
    key = np.asarray(template_np, np.float32).tobytes()
    if key not in _CACHE:
        meta, ktab, mtab, wblk = _prepare(template_np)
        nc = _build(meta, ktab.shape[1], mtab.shape[1])
        _CACHE[key] = (nc, meta, ktab, mtab, wblk)
    return _CACHE[key]


def kernel(hidden_states, lengths, keep_prob_template, image_grid_thw):
    global LAST_RESULTS
    hs = np.ascontiguousarray(np.asarray(hidden_states, np.float32))
    assert hs.shape == (B, N, D), hs.shape
    nc, meta, ktab, mtab, wblk = _get_program(keep_prob_template)

    kmtab = np.ascontiguousarray(np.concatenate([ktab, mtab], axis=1))
    in_maps = [
        {
            "h": hs[b],
            "kmidx": kmtab,
            "wblk": wblk,
        }
        for b in range(N_CORES)
    ]
    res = bass_utils.run_bass_kernel_spmd(
        nc, in_maps, core_ids=list(range(N_CORES))
    )
    LAST_RESULTS = res

    nk, nk_pad, C, n_halves = meta["nk"], meta["nk_pad"], meta["C"], meta["n_halves"]
    outs = []
    for b in range(N_CORES):
        o = res.results[b]["out"]
        parts = [o[:nk]]
        for hf in range(n_halves):
            r0 = nk_pad + hf * 128
            parts.append(o[r0 : r0 + min(128, C - hf * 128)])
        outs.append(np.concatenate(parts, axis=0))
    return np.concatenate(outs, axis=0)
